# revision 26
# baseline (speedup 1.0000x reference)
"""Causal multi-head attention mixer on 8 TRN2 NeuronCores — v8.

Sharding: core c -> (batch b = c//4, head-group hg = c%4 of 4 heads).
Host sums the 4 partial Wo outputs per batch (fp16 partials).

v8 vs v7:
  - DMA efficiency: x loaded as [128,1024] half-tiles (2KB partition
    lines), q/k/v weights host-packed to [128, 8*256] so each loads as
    two [128,1024] DMAs with 2KB lines. Rings run ~2x faster, killing
    the projection stalls in the first 50us.
  - Software pipelining across phases: fin(b-1) (rbc/normalize/Wo/out)
    and the V-projection matmuls of the next chunk are emitted as
    "filler" units woven between the scores and PV matmuls of each
    attention ki iteration. The attention inner loop is ACT(exp)-bound
    at ~1.1us/ki while PE only has ~0.65us of work; the woven units
    soak up the difference.
  - RoPE swap32 SBUF-SBUF DMAs moved off gpsimd's slow SWDGE ring to
    the sync+scalar hardware rings.
  - ACT: exp + qk eviction (runs in the QK phase where ACT is idle).
    DVE: everything else.
"""

import numpy as np
import ml_dtypes
from contextlib import ExitStack

import concourse.bass as bass
import concourse.tile as tile
from concourse import bacc, mybir
from concourse.bass_utils import run_bass_kernel_spmd

F32 = mybir.dt.float32
F32R = mybir.dt.float32r
F16 = mybir.dt.float16
BF16 = mybir.dt.bfloat16
CDT = BF16
AOP = mybir.AluOpType
AF = mybir.ActivationFunctionType

S = 2048          # seq len
DM = 1024         # model dim
HPC = 4           # heads per core
DH = 64           # head dim
CH = HPC * DH     # channels per core = 256
NCH = 4           # seq chunks (of 512)
QB = S // NCH     # 512
KT = 128          # k tile
NKT = S // KT     # 16
VB = DH + 1       # 65
ROPE_PERIOD = 10000.0


def _rope_tables():
    inv_freq = 1.0 / (ROPE_PERIOD ** (np.arange(0, DH, 2, dtype=np.float64) / DH))
    t = np.arange(S, dtype=np.float64)
    freqs = np.outer(inv_freq, t)           # [32, S]
    cos32 = np.cos(freqs)
    sin32 = np.sin(freqs)
    cos64 = np.concatenate([cos32, cos32], axis=0)          # [64, S]
    cosT = np.concatenate([cos64, cos64], axis=0)           # [128, S]
    sin64 = np.concatenate([sin32, -sin32], axis=0)         # [64, S]
    sinT2 = np.concatenate([sin64, sin64], axis=0)          # [128, S]
    return (cosT.astype(ml_dtypes.bfloat16), sinT2.astype(ml_dtypes.bfloat16))


def _build():
    nc = bacc.Bacc(None, target_bir_lowering=False)

    # x as 8 ktile x 2 half tensors is sliced from this [DM, S] layout
    xT_ext = nc.dram_tensor("xT", [DM, S], CDT, kind="ExternalInput")
    # weights pre-packed on host: [128, 8*CH], ktile kt at cols 256kt..
    wqP_ext = nc.dram_tensor("wqP", [128, 8 * CH], CDT, kind="ExternalInput")
    wkP_ext = nc.dram_tensor("wkP", [128, 8 * CH], CDT, kind="ExternalInput")
    wvP_ext = nc.dram_tensor("wvP", [128, 8 * CH], CDT, kind="ExternalInput")
    woT_ext = nc.dram_tensor("woT", [CH, DM], CDT, kind="ExternalInput")
    out_ext = nc.dram_tensor("out", [S, DM], F16, kind="ExternalOutput")

    cosT_np, sinT2_np = _rope_tables()
    cos_dram = nc.inline_tensor(cosT_np, name="cosW")
    sin_dram = nc.inline_tensor(sinT2_np, name="sinW")
    bm_np = np.where(np.arange(KT)[:, None] <= np.arange(KT)[None, :],
                     1.0, 0.0).astype(ml_dtypes.bfloat16)
    bm_dram = nc.inline_tensor(bm_np, name="bmask")
    onesr_np = np.ones((97, DH), dtype=np.float32)
    onesr_dram = nc.inline_tensor(onesr_np, name="onesr")

    with tile.TileContext(nc) as tc, ExitStack() as ctx:
        const = ctx.enter_context(tc.tile_pool(name="const", bufs=1))
        persist = ctx.enter_context(tc.tile_pool(name="persist", bufs=1))

        cosW = const.tile([128, S], CDT, tag="cosW")
        sinW = const.tile([128, S], CDT, tag="sinW")
        bmask = const.tile([KT, KT], CDT, tag="bmask")
        onesr = const.tile([97, DH], F32R, tag="onesr")

        # x: per-(ktile, half) tiles [128, 1024]; chunk cn lives in half
        # cn//2 at cols 512*(cn%2)..
        xh = [[const.tile([128, 2 * QB], CDT, name=f"x{k}_{h}")
               for h in range(2)] for k in range(8)]
        wqa = const.tile([128, 8 * CH], CDT, tag="wqa")
        wka = const.tile([128, 8 * CH], CDT, tag="wka")
        wva = const.tile([128, 8 * CH], CDT, tag="wva")
        wo_t = [const.tile([128, DM], CDT, name=f"wo{k}") for k in range(2)]

        # persistent activations
        qT_sb = [persist.tile([128, S], CDT, name=f"qT{m}") for m in range(2)]
        kp_sb = [persist.tile([128, S], CDT, name=f"kp{m}") for m in range(2)]
        attn_sb = [persist.tile([128, S], CDT, name=f"at{m}") for m in range(2)]
        v_sb = [persist.tile([128, HPC * VB], CDT, name=f"v{k}")
                for k in range(NKT)]

        # ---- DMA issuance ----
        # sync ring: weights for ktiles 0-3, first x tiles, weights 4-7,
        # rest — ordered so the kt-serial QK(0) accumulation can start as
        # early as possible
        HB = 8 * CH // 2  # 1024 cols per weight half
        nc.sync.dma_start(wqa[:, 0:HB], wqP_ext[:, 0:HB])
        nc.sync.dma_start(wka[:, 0:HB], wkP_ext[:, 0:HB])
        nc.sync.dma_start(xh[0][0][:], xT_ext[0:128, 0:2 * QB])
        nc.sync.dma_start(wqa[:, HB:2 * HB], wqP_ext[:, HB:2 * HB])
        nc.sync.dma_start(wka[:, HB:2 * HB], wkP_ext[:, HB:2 * HB])
        for k in (2, 4, 6):
            nc.sync.dma_start(xh[k][0][:], xT_ext[128 * k:128 * (k + 1),
                                                  0:2 * QB])
        for k in (1, 3, 5, 7):
            nc.sync.dma_start(xh[k][1][:], xT_ext[128 * k:128 * (k + 1),
                                                  2 * QB:4 * QB])
        # scalar ring: x-half0 odds first (QK0 critical), rope tables
        # before the last odd tile, then wv/mask/x-half1 evens
        for k in (1, 3):
            nc.scalar.dma_start(xh[k][0][:], xT_ext[128 * k:128 * (k + 1),
                                                    0:2 * QB])
        nc.scalar.dma_start(cosW[:], cos_dram[:])
        nc.scalar.dma_start(sinW[:], sin_dram[:])
        for k in (5, 7):
            nc.scalar.dma_start(xh[k][0][:], xT_ext[128 * k:128 * (k + 1),
                                                    0:2 * QB])
        nc.scalar.dma_start(wva[:, 0:HB], wvP_ext[:, 0:HB])
        nc.scalar.dma_start(wva[:, HB:2 * HB], wvP_ext[:, HB:2 * HB])
        nc.scalar.dma_start(bmask[:], bm_dram[:])
        for k in (0, 2, 4, 6):
            nc.scalar.dma_start(xh[k][1][:], xT_ext[128 * k:128 * (k + 1),
                                                    2 * QB:4 * QB])
        # gpsimd (SWDGE, slow): v ones-columns (pure memset), onesr, wo
        for k in range(NKT):
            vt3 = v_sb[k][:].rearrange("p (h e) -> p h e", h=HPC)
            nc.gpsimd.memset(vt3[:, :, DH:VB], 1.0)
        nc.gpsimd.dma_start(onesr[:], onesr_dram[:].bitcast(F32R))
        for k in range(2):
            nc.gpsimd.dma_start(wo_t[k][:], woT_ext[128 * k:128 * (k + 1), :])

        spool = ctx.enter_context(tc.tile_pool(name="spool", bufs=1, space="PSUM"))
        popool = ctx.enter_context(tc.tile_pool(name="popool", bufs=1, space="PSUM"))
        wpool = ctx.enter_context(tc.tile_pool(name="wpool", bufs=1, space="PSUM"))
        rpool = ctx.enter_context(tc.tile_pool(name="rpool", bufs=1))
        ptpool = ctx.enter_context(tc.tile_pool(name="ptpool", bufs=1))
        dpool = ctx.enter_context(tc.tile_pool(name="dpool", bufs=1))
        orawpool = ctx.enter_context(tc.tile_pool(name="orawpool", bufs=1))
        outpool = ctx.enter_context(tc.tile_pool(name="outpool", bufs=1))

        # preload the exp activation table
        warm = rpool.tile([1, 8], F32, tag="warm", bufs=1)
        nc.vector.memset(warm[:], 0.0)
        nc.scalar.activation(warm[:], warm[:], AF.Exp)

        # PE warmup fed by a memset tile (no DMA dependency): keeps the PE
        # active from the end of the preamble until real data arrives, so
        # HAM un-throttles before the first projections.
        warm_src = rpool.tile([128, 2 * CH], CDT, tag="wsrc", bufs=1)
        nc.vector.memset(warm_src[:], 0.25)
        warm_ps = wpool.tile([128, 512], F32, tag="w", bufs=2, name="warmps")
        for i in range(32):
            nc.tensor.matmul(warm_ps[:, 0:CH], warm_src[:, 0:128],
                             warm_src[:, 0:CH], start=True, stop=True)

        def wslice(wall, kt, m):
            return wall[:, CH * kt + 128 * m:CH * kt + 128 * (m + 1)]

        def xslice(cn, kt, lo=0, hi=QB):
            return xh[kt][cn // 2][:, QB * (cn % 2) + lo:QB * (cn % 2) + hi]

        def qk_rope(cn):
            """QK projection + RoPE for chunk cn (dense PE phase)."""
            cs = slice(QB * cn, QB * (cn + 1))
            for m in range(2):
                qk = spool.tile([128, 2 * QB], F32, tag="s", bufs=2,
                                name=f"qk{m}_{cn}")
                for kt in range(8):
                    nc.tensor.matmul(qk[:, 0:QB], wslice(wqa, kt, m),
                                     xslice(cn, kt), start=(kt == 0),
                                     stop=(kt == 7))
                    nc.tensor.matmul(qk[:, QB:2 * QB], wslice(wka, kt, m),
                                     xslice(cn, kt), start=(kt == 0),
                                     stop=(kt == 7))
                    if cn == 0 and m == 0:
                        # chunk 0 is DMA-paced: keep HAM warm between
                        # k-tile arrivals with filler matmuls
                        for i in range(3):
                            nc.tensor.matmul(warm_ps[:],
                                             warm_src[:, 0:128],
                                             warm_src[:],
                                             start=True, stop=True)
                # qk eviction on ACT (idle during this phase); RoPE on DVE
                qsb = rpool.tile([128, 2 * QB], CDT, tag="qsb", bufs=2)
                nc.scalar.copy(qsb[:], qk[:])
                qsb3 = qsb[:].rearrange("p (t q) -> p t q", t=2)
                rt = rpool.tile([128, 2 * QB], CDT, tag="rt", bufs=2)
                rst = rpool.tile([128, 2 * QB], CDT, tag="rst", bufs=2)
                tt = rpool.tile([128, 2 * QB], CDT, tag="tt", bufs=2)
                rt3 = rt[:].rearrange("p (t q) -> p t q", t=2)
                tt3 = tt[:].rearrange("p (t q) -> p t q", t=2)
                sinb = sinW[:, cs].unsqueeze(1).broadcast_to([128, 2, QB])
                cosb = cosW[:, cs].unsqueeze(1).broadcast_to([128, 2, QB])
                nc.vector.tensor_tensor(rt3, qsb3, sinb, AOP.mult)
                for blk, eng in zip(range(4), (nc.sync, nc.scalar,
                                               nc.scalar, nc.sync)):
                    src = slice(32 * (blk ^ 1), 32 * (blk ^ 1) + 32)
                    dst_sl = slice(32 * blk, 32 * blk + 32)
                    eng.dma_start(rst[dst_sl, :], rt[src, :])
                nc.vector.tensor_tensor(tt3, qsb3, cosb, AOP.mult)
                nc.vector.tensor_tensor(qT_sb[m][:, cs], tt[:, 0:QB],
                                        rst[:, 0:QB], AOP.add)
                nc.vector.tensor_tensor(kp_sb[m][:, cs], tt[:, QB:2 * QB],
                                        rst[:, QB:2 * QB], AOP.add)

        def v_fillers(cn):
            """V projection for chunk cn as a list of filler units."""
            state = {}

            def mk_mm(kt, sq):
                def f():
                    if "ps" not in state:
                        state["ps"] = [
                            wpool.tile([128, 2 * CH], F32, tag="w", bufs=2,
                                       name=f"vp{j}_{cn}") for j in range(2)]
                    v_ps = state["ps"]
                    nc.tensor.matmul(
                        v_ps[sq // 2][:, CH * (sq % 2):CH * (sq % 2 + 1)],
                        xslice(cn, kt, 128 * sq, 128 * (sq + 1)),
                        wva[:, CH * kt:CH * (kt + 1)],
                        start=(kt == 0 and sq % 2 == 0),
                        stop=(kt == 7 and sq % 2 == 1))
                return f

            def mk_ev(sq):
                def f():
                    v_ps = state["ps"]
                    vt3 = v_sb[4 * cn + sq][:].rearrange(
                        "p (h e) -> p h e", h=HPC)
                    vsrc = v_ps[sq // 2][:, CH * (sq % 2):CH * (sq % 2 + 1)]
                    nc.vector.tensor_copy(
                        vt3[:, :, 0:DH],
                        vsrc.rearrange("p (h d) -> p h d", h=HPC))
                return f

            fs = []
            # 2 matmuls per filler unit
            pend = []
            for kt in range(8):
                for sq in range(4):
                    pend.append(mk_mm(kt, sq))
                    if len(pend) == 2:
                        fs.append((lambda ps=tuple(pend):
                                   [p() for p in ps]))
                        pend = []
            for sq in range(4):
                fs.append(mk_ev(sq))
            return fs

        blk_state = {}

        def fin_fillers(b):
            """fin(b): recip, rbc+normalize, Wo, out-DMA as filler units."""
            qs = slice(QB * b, QB * (b + 1))
            dpack, opairs = blk_state.pop(b)
            st = {}

            def recip():
                drec = dpool.tile([97, QB], F32, tag="drec", bufs=2,
                                  name=f"dr{b}")
                nc.vector.reciprocal_approx_fast(drec[:], dpack[:])
                drecr = dpool.tile([97, QB], F32R, tag="drecr", bufs=2,
                                   name=f"drr{b}")
                with nc.allow_low_precision(
                        reason="f32r rows feed PE broadcast"):
                    nc.vector.tensor_copy(drecr[:], drec[:])
                st["drecr"] = drecr

            def mk_rbc_pair(m):
                def f():
                    # two rank-1 broadcasts on disjoint row groups run
                    # concurrently on the PE
                    drecr = st["drecr"]
                    for hh in range(2):
                        r = 2 * m + hh
                        rbc = wpool.tile([128, QB], F32, tag="w", bufs=2,
                                         name=f"rbc{r}_{b}")
                        nc.tensor.matmul(
                            rbc[0:DH, :], onesr[32 * r:32 * r + 1, :],
                            drecr[32 * r:32 * r + 1, :],
                            start=True, stop=True,
                            tile_position=(32 * r, 0))
                        st[("rbc", m, hh)] = rbc
                return f

            def mk_norm(m, hh):
                def f():
                    rbc = st[("rbc", m, hh)]
                    hrow = slice(64 * hh, 64 * hh + 64)
                    nc.vector.tensor_tensor(attn_sb[m][hrow, qs],
                                            opairs[m][hrow, :],
                                            rbc[0:DH, :], AOP.mult)
                return f

            def mk_wo(sq, on):
                def f():
                    if sq not in st:
                        st[sq] = outpool.tile([128, DM], F16, tag="ot",
                                              bufs=4, name=f"ot{sq}")
                    ot = st[sq]
                    osl = slice(512 * on, 512 * (on + 1))
                    ssl = slice(128 * sq, 128 * (sq + 1))
                    ops = wpool.tile([128, 512], F32, tag="w", bufs=2,
                                     name=f"wo{sq}_{on}")
                    for ct in range(2):
                        nc.tensor.matmul(ops[:], attn_sb[ct][:, ssl],
                                         wo_t[ct][:, osl],
                                         start=(ct == 0), stop=(ct == 1))
                    nc.vector.tensor_copy(ot[:, osl], ops[:])
                return f

            def mk_out(sq):
                def f():
                    ssl = slice(128 * sq, 128 * (sq + 1))
                    if b == 3:
                        # tail: only the fast hardware rings
                        eng = (nc.sync, nc.scalar, nc.sync,
                               nc.scalar)[sq % 4]
                    else:
                        eng = nc.sync if sq % 2 == 0 else nc.gpsimd
                    eng.dma_start(out_ext[ssl, :], st[sq][:])
                return f

            fs = [recip]
            for m in range(2):
                fs.append(mk_rbc_pair(m))
                for hh in range(2):
                    fs.append(mk_norm(m, hh))
            for sq in range(4 * b, 4 * b + 4):
                for on in range(2):
                    fs.append(mk_wo(sq, on))
                fs.append(mk_out(sq))
            return fs

        def attention_block(b, fillers):
            nkt = 4 * b + 4
            fi = [0]

            def fill(n):
                while n > 0 and fi[0] < len(fillers):
                    fillers[fi[0]]()
                    fi[0] += 1
                    n -= 1

            dpack = dpool.tile([97, QB], F32, tag="dpack", bufs=4,
                               name=f"dp{b}")
            nc.gpsimd.memset(dpack[:], 1.0)
            opairs = []
            for m in range(2):
                po = [popool.tile([VB, QB], F32, tag="po", bufs=2,
                                  name=f"po{hh}_{m}_{b}") for hh in range(2)]
                for ki in range(nkt):
                    d = ki - 4 * b
                    qlo = max(0, 128 * d)
                    s_t = spool.tile([128, 2 * QB], F32, tag="s", bufs=2,
                                     name=f"sc_{m}_{b}_{ki}")
                    for hh in range(2):
                        hr = slice(64 * hh, 64 * hh + 64)
                        nc.tensor.matmul(
                            s_t[:, QB * hh + qlo:QB * (hh + 1)],
                            kp_sb[m][hr, 128 * ki:128 * (ki + 1)],
                            qT_sb[m][hr, QB * b + qlo:QB * (b + 1)],
                            start=True, stop=True)
                    sc3 = s_t[:].rearrange("p (g q) -> p g q", g=2)
                    p_t = ptpool.tile([128, 2 * QB], CDT, tag="pt", bufs=3,
                                      name=f"pt_{m}_{b}_{ki}")
                    pt3 = p_t[:].rearrange("p (g q) -> p g q", g=2)
                    nc.scalar.activation(pt3[:, :, qlo:QB], sc3[:, :, qlo:QB],
                                         AF.Exp, scale=0.125)
                    if d >= 0:
                        nc.vector.tensor_tensor(
                            pt3[:, :, qlo:qlo + 128],
                            pt3[:, :, qlo:qlo + 128],
                            bmask[:].unsqueeze(1).broadcast_to([128, 2, 128]),
                            AOP.mult)
                    fill(1)
                    for hh in range(2):
                        h = 2 * m + hh
                        nc.tensor.matmul(
                            po[hh][:, qlo:QB],
                            v_sb[ki][:, VB * h:VB * h + VB],
                            p_t[:, QB * hh + qlo:QB * (hh + 1)],
                            start=(ki == 0), stop=(ki == nkt - 1))
                # evict o pair + denominator rows (DVE)
                opair = orawpool.tile([128, QB], F32, tag="opair", bufs=8,
                                      name=f"op{m}_{b}")
                for hh in range(2):
                    nc.vector.tensor_copy(opair[64 * hh:64 * hh + 64, :],
                                          po[hh][0:DH, :])
                    nc.vector.tensor_copy(dpack[32 * (2 * m + hh):
                                                32 * (2 * m + hh) + 1, :],
                                          po[hh][DH:DH + 1, :])
                opairs.append(opair)
            blk_state[b] = (dpack, opairs)
            fill(len(fillers))  # drain leftovers

        # ---- schedule ----
        # fin(b) has no downstream consumers (Wo partials just DMA out),
        # so its PE work is deferred into the LATE attention blocks where
        # exp dominates and the PE has idle slots.
        qk_rope(0)
        # V(0) dense (overlaps the x-half0 DMA tail)
        for f in v_fillers(0):
            f()
        attention_block(0, v_fillers(1))
        qk_rope(1)
        attention_block(1, v_fillers(2))
        qk_rope(2)
        attention_block(2, fin_fillers(0) + v_fillers(3))
        qk_rope(3)
        attention_block(3, fin_fillers(1) + fin_fillers(2))
        for f in fin_fillers(3):
            f()

    nc.compile()
    return nc


def _pack_w(wT):
    # wT: [DM, CH] -> [128, 8*CH]: ktile kt -> cols CH*kt..CH*(kt+1)
    return np.ascontiguousarray(
        wT.reshape(8, 128, CH).transpose(1, 0, 2).reshape(128, 8 * CH))


def _in_maps(x, Wq, Wk, Wv, Wo):
    x = np.asarray(x, dtype=np.float32)
    Wq = np.asarray(Wq, dtype=np.float32)
    Wk = np.asarray(Wk, dtype=np.float32)
    Wv = np.asarray(Wv, dtype=np.float32)
    Wo = np.asarray(Wo, dtype=np.float32)
    np_cdt = ml_dtypes.bfloat16
    in_maps = []
    for c in range(8):
        b, hg = divmod(c, 4)
        rows = slice(CH * hg, CH * (hg + 1))
        in_maps.append({
            "xT": np.ascontiguousarray(x[b].T).astype(np_cdt),
            "wqP": _pack_w(Wq[rows, :].T).astype(np_cdt),
            "wkP": _pack_w(Wk[rows, :].T).astype(np_cdt),
            "wvP": _pack_w(Wv[rows, :].T).astype(np_cdt),
            "woT": np.ascontiguousarray(Wo[:, rows].T).astype(np_cdt),
        })
    return in_maps


_NC_CACHE = []


def kernel(x, Wq, Wk, Wv, Wo):
    in_maps = _in_maps(x, Wq, Wk, Wv, Wo)
    if not _NC_CACHE:
        _NC_CACHE.append(_build())
    nc = _NC_CACHE[0]

    res = run_bass_kernel_spmd(nc, in_maps, list(range(8)))
    out = np.zeros((2, S, DM), dtype=np.float32)
    for c in range(8):
        out[c // 4] += res.results[c]["out"].astype(np.float32)
    return out


# revision 29
# speedup vs baseline: 1.0127x; 1.0127x over previous
"""Causal multi-head attention mixer on 8 TRN2 NeuronCores — v8.

Sharding: core c -> (batch b = c//4, head-group hg = c%4 of 4 heads).
Host sums the 4 partial Wo outputs per batch (fp16 partials).

v8 vs v7:
  - DMA efficiency: x loaded as [128,1024] half-tiles (2KB partition
    lines), q/k/v weights host-packed to [128, 8*256] so each loads as
    two [128,1024] DMAs with 2KB lines. Rings run ~2x faster, killing
    the projection stalls in the first 50us.
  - Software pipelining across phases: fin(b-1) (rbc/normalize/Wo/out)
    and the V-projection matmuls of the next chunk are emitted as
    "filler" units woven between the scores and PV matmuls of each
    attention ki iteration. The attention inner loop is ACT(exp)-bound
    at ~1.1us/ki while PE only has ~0.65us of work; the woven units
    soak up the difference.
  - RoPE swap32 SBUF-SBUF DMAs moved off gpsimd's slow SWDGE ring to
    the sync+scalar hardware rings.
  - ACT: exp + qk eviction (runs in the QK phase where ACT is idle).
    DVE: everything else.
"""

import numpy as np
import ml_dtypes
from contextlib import ExitStack

import concourse.bass as bass
import concourse.tile as tile
from concourse import bacc, mybir
from concourse.bass_utils import run_bass_kernel_spmd

F32 = mybir.dt.float32
F32R = mybir.dt.float32r
F16 = mybir.dt.float16
BF16 = mybir.dt.bfloat16
CDT = BF16
AOP = mybir.AluOpType
AF = mybir.ActivationFunctionType

S = 2048          # seq len
DM = 1024         # model dim
HPC = 4           # heads per core
DH = 64           # head dim
CH = HPC * DH     # channels per core = 256
NCH = 4           # seq chunks (of 512)
QB = S // NCH     # 512
KT = 128          # k tile
NKT = S // KT     # 16
VB = DH + 1       # 65
ROPE_PERIOD = 10000.0


def _rope_tables():
    inv_freq = 1.0 / (ROPE_PERIOD ** (np.arange(0, DH, 2, dtype=np.float64) / DH))
    t = np.arange(S, dtype=np.float64)
    freqs = np.outer(inv_freq, t)           # [32, S]
    cos32 = np.cos(freqs)
    sin32 = np.sin(freqs)
    cos64 = np.concatenate([cos32, cos32], axis=0)          # [64, S]
    cosT = np.concatenate([cos64, cos64], axis=0)           # [128, S]
    sin64 = np.concatenate([sin32, -sin32], axis=0)         # [64, S]
    sinT2 = np.concatenate([sin64, sin64], axis=0)          # [128, S]
    return (cosT.astype(ml_dtypes.bfloat16), sinT2.astype(ml_dtypes.bfloat16))


def _build():
    nc = bacc.Bacc(None, target_bir_lowering=False)

    # x as 8 ktile x 2 half tensors is sliced from this [DM, S] layout
    xT_ext = nc.dram_tensor("xT", [DM, S], CDT, kind="ExternalInput")
    # weights pre-packed on host: [128, 8*CH], ktile kt at cols 256kt..
    wqP_ext = nc.dram_tensor("wqP", [128, 8 * CH], CDT, kind="ExternalInput")
    wkP_ext = nc.dram_tensor("wkP", [128, 8 * CH], CDT, kind="ExternalInput")
    wvP_ext = nc.dram_tensor("wvP", [128, 8 * CH], CDT, kind="ExternalInput")
    woT_ext = nc.dram_tensor("woT", [CH, DM], CDT, kind="ExternalInput")
    out_ext = nc.dram_tensor("out", [S, DM], F16, kind="ExternalOutput")

    cosT_np, sinT2_np = _rope_tables()
    cos_dram = nc.inline_tensor(cosT_np, name="cosW")
    sin_dram = nc.inline_tensor(sinT2_np, name="sinW")
    bm_np = np.where(np.arange(KT)[:, None] <= np.arange(KT)[None, :],
                     1.0, 0.0).astype(ml_dtypes.bfloat16)
    bm_dram = nc.inline_tensor(bm_np, name="bmask")
    onesr_np = np.ones((97, DH), dtype=np.float32)
    onesr_dram = nc.inline_tensor(onesr_np, name="onesr")

    with tile.TileContext(nc) as tc, ExitStack() as ctx:
        const = ctx.enter_context(tc.tile_pool(name="const", bufs=1))
        persist = ctx.enter_context(tc.tile_pool(name="persist", bufs=1))

        cosW = const.tile([128, S], CDT, tag="cosW")
        sinW = const.tile([128, S], CDT, tag="sinW")
        bmask = const.tile([KT, KT], CDT, tag="bmask")
        onesr = const.tile([97, DH], F32R, tag="onesr")

        # x: per-(ktile, half) tiles [128, 1024]; chunk cn lives in half
        # cn//2 at cols 512*(cn%2)..
        xh = [[const.tile([128, 2 * QB], CDT, name=f"x{k}_{h}")
               for h in range(2)] for k in range(8)]
        wqa = const.tile([128, 8 * CH], CDT, tag="wqa")
        wka = const.tile([128, 8 * CH], CDT, tag="wka")
        wva = const.tile([128, 8 * CH], CDT, tag="wva")
        wo_t = [const.tile([128, DM], CDT, name=f"wo{k}") for k in range(2)]

        # persistent activations
        qT_sb = [persist.tile([128, S], CDT, name=f"qT{m}") for m in range(2)]
        kp_sb = [persist.tile([128, S], CDT, name=f"kp{m}") for m in range(2)]
        attn_sb = [persist.tile([128, S], CDT, name=f"at{m}") for m in range(2)]
        v_sb = [persist.tile([128, HPC * VB], CDT, name=f"v{k}")
                for k in range(NKT)]

        # ---- DMA issuance ----
        # sync ring: weights for ktiles 0-3, first x tiles, weights 4-7,
        # rest — ordered so the kt-serial QK(0) accumulation can start as
        # early as possible
        HB = 8 * CH // 2  # 1024 cols per weight half
        nc.sync.dma_start(wqa[:, 0:HB], wqP_ext[:, 0:HB])
        nc.sync.dma_start(wka[:, 0:HB], wkP_ext[:, 0:HB])
        nc.sync.dma_start(xh[0][0][:], xT_ext[0:128, 0:2 * QB])
        nc.sync.dma_start(wqa[:, HB:2 * HB], wqP_ext[:, HB:2 * HB])
        nc.sync.dma_start(wka[:, HB:2 * HB], wkP_ext[:, HB:2 * HB])
        for k in (2, 4, 6):
            nc.sync.dma_start(xh[k][0][:], xT_ext[128 * k:128 * (k + 1),
                                                  0:2 * QB])
        for k in (1, 3, 5, 7):
            nc.sync.dma_start(xh[k][1][:], xT_ext[128 * k:128 * (k + 1),
                                                  2 * QB:4 * QB])
        # scalar ring: x-half0 odds first (QK0 critical), rope tables
        # before the last odd tile, then wv/mask/x-half1 evens
        for k in (1, 3):
            nc.scalar.dma_start(xh[k][0][:], xT_ext[128 * k:128 * (k + 1),
                                                    0:2 * QB])
        nc.scalar.dma_start(cosW[:], cos_dram[:])
        nc.scalar.dma_start(sinW[:], sin_dram[:])
        for k in (5, 7):
            nc.scalar.dma_start(xh[k][0][:], xT_ext[128 * k:128 * (k + 1),
                                                    0:2 * QB])
        nc.scalar.dma_start(wva[:, 0:HB], wvP_ext[:, 0:HB])
        nc.scalar.dma_start(wva[:, HB:2 * HB], wvP_ext[:, HB:2 * HB])
        nc.scalar.dma_start(bmask[:], bm_dram[:])
        for k in (0, 2, 4, 6):
            nc.scalar.dma_start(xh[k][1][:], xT_ext[128 * k:128 * (k + 1),
                                                    2 * QB:4 * QB])
        # gpsimd (SWDGE, slow): v ones-columns (pure memset), onesr, wo
        for k in range(NKT):
            vt3 = v_sb[k][:].rearrange("p (h e) -> p h e", h=HPC)
            nc.gpsimd.memset(vt3[:, :, DH:VB], 1.0)
        nc.gpsimd.dma_start(onesr[:], onesr_dram[:].bitcast(F32R))
        for k in range(2):
            nc.gpsimd.dma_start(wo_t[k][:], woT_ext[128 * k:128 * (k + 1), :])

        spool = ctx.enter_context(tc.tile_pool(name="spool", bufs=1, space="PSUM"))
        popool = ctx.enter_context(tc.tile_pool(name="popool", bufs=1, space="PSUM"))
        wpool = ctx.enter_context(tc.tile_pool(name="wpool", bufs=1, space="PSUM"))
        rpool = ctx.enter_context(tc.tile_pool(name="rpool", bufs=1))
        ptpool = ctx.enter_context(tc.tile_pool(name="ptpool", bufs=1))
        dpool = ctx.enter_context(tc.tile_pool(name="dpool", bufs=1))
        orawpool = ctx.enter_context(tc.tile_pool(name="orawpool", bufs=1))
        outpool = ctx.enter_context(tc.tile_pool(name="outpool", bufs=1))

        # preload the exp activation table
        warm = rpool.tile([1, 8], F32, tag="warm", bufs=1)
        nc.vector.memset(warm[:], 0.0)
        nc.scalar.activation(warm[:], warm[:], AF.Exp)

        # PE warmup fed by a memset tile (no DMA dependency): keeps the PE
        # active from the end of the preamble until real data arrives, so
        # HAM un-throttles before the first projections.
        warm_src = rpool.tile([128, 2 * CH], CDT, tag="wsrc", bufs=1)
        nc.vector.memset(warm_src[:], 0.25)
        warm_ps = wpool.tile([128, 512], F32, tag="w", bufs=2, name="warmps")
        for i in range(32):
            nc.tensor.matmul(warm_ps[:, 0:CH], warm_src[:, 0:128],
                             warm_src[:, 0:CH], start=True, stop=True)

        def wslice(wall, kt, m):
            return wall[:, CH * kt + 128 * m:CH * kt + 128 * (m + 1)]

        def xslice(cn, kt, lo=0, hi=QB):
            return xh[kt][cn // 2][:, QB * (cn % 2) + lo:QB * (cn % 2) + hi]

        def qk_rope(cn):
            """QK projection + RoPE for chunk cn (dense PE phase)."""
            cs = slice(QB * cn, QB * (cn + 1))
            for m in range(2):
                qk = spool.tile([128, 2 * QB], F32, tag="s", bufs=2,
                                name=f"qk{m}_{cn}")
                for kt in range(8):
                    nc.tensor.matmul(qk[:, 0:QB], wslice(wqa, kt, m),
                                     xslice(cn, kt), start=(kt == 0),
                                     stop=(kt == 7))
                    nc.tensor.matmul(qk[:, QB:2 * QB], wslice(wka, kt, m),
                                     xslice(cn, kt), start=(kt == 0),
                                     stop=(kt == 7))

                # qk eviction on ACT (idle during this phase); RoPE on DVE
                qsb = rpool.tile([128, 2 * QB], CDT, tag="qsb", bufs=2)
                nc.scalar.copy(qsb[:], qk[:])
                qsb3 = qsb[:].rearrange("p (t q) -> p t q", t=2)
                rt = rpool.tile([128, 2 * QB], CDT, tag="rt", bufs=2)
                rst = rpool.tile([128, 2 * QB], CDT, tag="rst", bufs=2)
                tt = rpool.tile([128, 2 * QB], CDT, tag="tt", bufs=2)
                rt3 = rt[:].rearrange("p (t q) -> p t q", t=2)
                tt3 = tt[:].rearrange("p (t q) -> p t q", t=2)
                sinb = sinW[:, cs].unsqueeze(1).broadcast_to([128, 2, QB])
                cosb = cosW[:, cs].unsqueeze(1).broadcast_to([128, 2, QB])
                nc.vector.tensor_tensor(rt3, qsb3, sinb, AOP.mult)
                for blk, eng in zip(range(4), (nc.sync, nc.scalar,
                                               nc.scalar, nc.sync)):
                    src = slice(32 * (blk ^ 1), 32 * (blk ^ 1) + 32)
                    dst_sl = slice(32 * blk, 32 * blk + 32)
                    eng.dma_start(rst[dst_sl, :], rt[src, :])
                nc.vector.tensor_tensor(tt3, qsb3, cosb, AOP.mult)
                nc.vector.tensor_tensor(qT_sb[m][:, cs], tt[:, 0:QB],
                                        rst[:, 0:QB], AOP.add)
                nc.vector.tensor_tensor(kp_sb[m][:, cs], tt[:, QB:2 * QB],
                                        rst[:, QB:2 * QB], AOP.add)

        def v_fillers(cn):
            """V projection for chunk cn as a list of filler units."""
            state = {}

            def mk_mm(kt, sq):
                def f():
                    if "ps" not in state:
                        state["ps"] = [
                            wpool.tile([128, 2 * CH], F32, tag="w", bufs=2,
                                       name=f"vp{j}_{cn}") for j in range(2)]
                    v_ps = state["ps"]
                    nc.tensor.matmul(
                        v_ps[sq // 2][:, CH * (sq % 2):CH * (sq % 2 + 1)],
                        xslice(cn, kt, 128 * sq, 128 * (sq + 1)),
                        wva[:, CH * kt:CH * (kt + 1)],
                        start=(kt == 0 and sq % 2 == 0),
                        stop=(kt == 7 and sq % 2 == 1))
                return f

            def mk_ev(sq):
                def f():
                    v_ps = state["ps"]
                    vt3 = v_sb[4 * cn + sq][:].rearrange(
                        "p (h e) -> p h e", h=HPC)
                    vsrc = v_ps[sq // 2][:, CH * (sq % 2):CH * (sq % 2 + 1)]
                    nc.vector.tensor_copy(
                        vt3[:, :, 0:DH],
                        vsrc.rearrange("p (h d) -> p h d", h=HPC))
                return f

            fs = []
            # 2 matmuls per filler unit
            pend = []
            for kt in range(8):
                for sq in range(4):
                    pend.append(mk_mm(kt, sq))
                    if len(pend) == 2:
                        fs.append((lambda ps=tuple(pend):
                                   [p() for p in ps]))
                        pend = []
            for sq in range(4):
                fs.append(mk_ev(sq))
            return fs

        blk_state = {}

        def fin_fillers(b):
            """fin(b): recip, rbc+normalize, Wo, out-DMA as filler units."""
            qs = slice(QB * b, QB * (b + 1))
            dpack, opairs = blk_state.pop(b)
            st = {}

            def recip():
                drec = dpool.tile([97, QB], F32, tag="drec", bufs=2,
                                  name=f"dr{b}")
                nc.vector.reciprocal_approx_fast(drec[:], dpack[:])
                drecr = dpool.tile([97, QB], F32R, tag="drecr", bufs=2,
                                   name=f"drr{b}")
                with nc.allow_low_precision(
                        reason="f32r rows feed PE broadcast"):
                    nc.vector.tensor_copy(drecr[:], drec[:])
                st["drecr"] = drecr

            def mk_rbc_pair(m):
                def f():
                    # two rank-1 broadcasts on disjoint row groups run
                    # concurrently on the PE
                    drecr = st["drecr"]
                    for hh in range(2):
                        r = 2 * m + hh
                        rbc = wpool.tile([128, QB], F32, tag="w", bufs=2,
                                         name=f"rbc{r}_{b}")
                        nc.tensor.matmul(
                            rbc[0:DH, :], onesr[32 * r:32 * r + 1, :],
                            drecr[32 * r:32 * r + 1, :],
                            start=True, stop=True,
                            tile_position=(32 * r, 0))
                        st[("rbc", m, hh)] = rbc
                return f

            def mk_norm(m, hh):
                def f():
                    rbc = st[("rbc", m, hh)]
                    hrow = slice(64 * hh, 64 * hh + 64)
                    nc.vector.tensor_tensor(attn_sb[m][hrow, qs],
                                            opairs[m][hrow, :],
                                            rbc[0:DH, :], AOP.mult)
                return f

            def mk_wo(sq, on):
                def f():
                    if sq not in st:
                        st[sq] = outpool.tile([128, DM], F16, tag="ot",
                                              bufs=4, name=f"ot{sq}")
                    ot = st[sq]
                    osl = slice(512 * on, 512 * (on + 1))
                    ssl = slice(128 * sq, 128 * (sq + 1))
                    ops = wpool.tile([128, 512], F32, tag="w", bufs=2,
                                     name=f"wo{sq}_{on}")
                    for ct in range(2):
                        nc.tensor.matmul(ops[:], attn_sb[ct][:, ssl],
                                         wo_t[ct][:, osl],
                                         start=(ct == 0), stop=(ct == 1))
                    nc.vector.tensor_copy(ot[:, osl], ops[:])
                return f

            def mk_out(sq):
                def f():
                    ssl = slice(128 * sq, 128 * (sq + 1))
                    if b == 3:
                        # tail: only the fast hardware rings
                        eng = (nc.sync, nc.scalar, nc.sync,
                               nc.scalar)[sq % 4]
                    else:
                        eng = nc.sync if sq % 2 == 0 else nc.gpsimd
                    eng.dma_start(out_ext[ssl, :], st[sq][:])
                return f

            fs = [recip]
            for m in range(2):
                fs.append(mk_rbc_pair(m))
                for hh in range(2):
                    fs.append(mk_norm(m, hh))
            for sq in range(4 * b, 4 * b + 4):
                for on in range(2):
                    fs.append(mk_wo(sq, on))
                fs.append(mk_out(sq))
            return fs

        def attention_block(b, fillers):
            nkt = 4 * b + 4
            fi = [0]

            def fill(n):
                while n > 0 and fi[0] < len(fillers):
                    fillers[fi[0]]()
                    fi[0] += 1
                    n -= 1

            dpack = dpool.tile([97, QB], F32, tag="dpack", bufs=4,
                               name=f"dp{b}")
            nc.gpsimd.memset(dpack[:], 1.0)
            opairs = []
            for m in range(2):
                po = [popool.tile([VB, QB], F32, tag="po", bufs=2,
                                  name=f"po{hh}_{m}_{b}") for hh in range(2)]
                for ki in range(nkt):
                    d = ki - 4 * b
                    qlo = max(0, 128 * d)
                    s_t = spool.tile([128, 2 * QB], F32, tag="s", bufs=2,
                                     name=f"sc_{m}_{b}_{ki}")
                    for hh in range(2):
                        hr = slice(64 * hh, 64 * hh + 64)
                        nc.tensor.matmul(
                            s_t[:, QB * hh + qlo:QB * (hh + 1)],
                            kp_sb[m][hr, 128 * ki:128 * (ki + 1)],
                            qT_sb[m][hr, QB * b + qlo:QB * (b + 1)],
                            start=True, stop=True)
                    sc3 = s_t[:].rearrange("p (g q) -> p g q", g=2)
                    p_t = ptpool.tile([128, 2 * QB], CDT, tag="pt", bufs=3,
                                      name=f"pt_{m}_{b}_{ki}")
                    pt3 = p_t[:].rearrange("p (g q) -> p g q", g=2)
                    nc.scalar.activation(pt3[:, :, qlo:QB], sc3[:, :, qlo:QB],
                                         AF.Exp, scale=0.125)
                    if d >= 0:
                        nc.vector.tensor_tensor(
                            pt3[:, :, qlo:qlo + 128],
                            pt3[:, :, qlo:qlo + 128],
                            bmask[:].unsqueeze(1).broadcast_to([128, 2, 128]),
                            AOP.mult)
                    fill(1)
                    for hh in range(2):
                        h = 2 * m + hh
                        nc.tensor.matmul(
                            po[hh][:, qlo:QB],
                            v_sb[ki][:, VB * h:VB * h + VB],
                            p_t[:, QB * hh + qlo:QB * (hh + 1)],
                            start=(ki == 0), stop=(ki == nkt - 1))
                # evict o pair + denominator rows (DVE)
                opair = orawpool.tile([128, QB], F32, tag="opair", bufs=8,
                                      name=f"op{m}_{b}")
                for hh in range(2):
                    nc.vector.tensor_copy(opair[64 * hh:64 * hh + 64, :],
                                          po[hh][0:DH, :])
                    nc.vector.tensor_copy(dpack[32 * (2 * m + hh):
                                                32 * (2 * m + hh) + 1, :],
                                          po[hh][DH:DH + 1, :])
                opairs.append(opair)
            blk_state[b] = (dpack, opairs)
            fill(len(fillers))  # drain leftovers

        # ---- schedule ----
        # fin(b) has no downstream consumers (Wo partials just DMA out),
        # so its PE work is deferred into the LATE attention blocks where
        # exp dominates and the PE has idle slots.
        qk_rope(0)
        # V(0) dense (overlaps the x-half0 DMA tail)
        for f in v_fillers(0):
            f()
        attention_block(0, v_fillers(1))
        qk_rope(1)
        attention_block(1, v_fillers(2))
        bridge_no = [0]

        def ham_bridge(n):
            # the PE idles behind a DVE chain here; free matmuls keep the
            # HAM activity window busy so the next phase starts at 2.4GHz
            bridge_no[0] += 1
            wb = wpool.tile([128, 512], F32, tag="w", bufs=2,
                            name=f"hb{bridge_no[0]}")
            for i in range(n):
                nc.tensor.matmul(wb[:], warm_src[:, 0:128],
                                 warm_src[:], start=True, stop=True)

        qk_rope(2)
        attention_block(2, fin_fillers(0) + v_fillers(3))
        qk_rope(3)
        ham_bridge(8)
        attention_block(3, fin_fillers(1) + fin_fillers(2))
        ham_bridge(10)
        for f in fin_fillers(3):
            f()

    nc.compile()
    return nc


def _pack_w(wT):
    # wT: [DM, CH] -> [128, 8*CH]: ktile kt -> cols CH*kt..CH*(kt+1)
    return np.ascontiguousarray(
        wT.reshape(8, 128, CH).transpose(1, 0, 2).reshape(128, 8 * CH))


def _in_maps(x, Wq, Wk, Wv, Wo):
    x = np.asarray(x, dtype=np.float32)
    Wq = np.asarray(Wq, dtype=np.float32)
    Wk = np.asarray(Wk, dtype=np.float32)
    Wv = np.asarray(Wv, dtype=np.float32)
    Wo = np.asarray(Wo, dtype=np.float32)
    np_cdt = ml_dtypes.bfloat16
    in_maps = []
    for c in range(8):
        b, hg = divmod(c, 4)
        rows = slice(CH * hg, CH * (hg + 1))
        in_maps.append({
            "xT": np.ascontiguousarray(x[b].T).astype(np_cdt),
            "wqP": _pack_w(Wq[rows, :].T).astype(np_cdt),
            "wkP": _pack_w(Wk[rows, :].T).astype(np_cdt),
            "wvP": _pack_w(Wv[rows, :].T).astype(np_cdt),
            "woT": np.ascontiguousarray(Wo[:, rows].T).astype(np_cdt),
        })
    return in_maps


_NC_CACHE = []


def kernel(x, Wq, Wk, Wv, Wo):
    in_maps = _in_maps(x, Wq, Wk, Wv, Wo)
    if not _NC_CACHE:
        _NC_CACHE.append(_build())
    nc = _NC_CACHE[0]

    res = run_bass_kernel_spmd(nc, in_maps, list(range(8)))
    out = np.zeros((2, S, DM), dtype=np.float32)
    for c in range(8):
        out[c // 4] += res.results[c]["out"].astype(np.float32)
    return out
